# revision 1
# baseline (speedup 1.0000x reference)
"""Masked phase-locking value (PLV) kernel for Trainium2, 8 NeuronCores.

Math: out[b] = |sum_ij M_ij * exp(i*(a_bi - b_bj))| / max(sum(M), 1)
    real_b = sum_ij M_ij (cos a_bi cos b_bj + sin a_bi sin b_bj)
    imag_b = sum_ij M_ij (sin a_bi cos b_bj - cos a_bi sin b_bj)

Device decomposition (per core, Na sharded 8 ways -> 1024 rows each):
    acc[m, j] = sum_i W[i, m] * mask[i, j]     (TensorE; W = [ca^T | sa^T], m = 2B = 128)
    racc[m]   = sum_j acc[m, j] * CS[m, j]     (DVE mult, ACT accumulate; CS = [cb; sb])
    qacc[m]   = sum_j acc[m, j] * SW[m, j]     (SW = [sb; cb], partition-swap of CS)
real_b = sum_cores racc[b] + racc[64+b]; imag_b = sum_cores qacc[64+b] - qacc[b].
All bilinear in mask rows, so Na-shard partials just add; host does the tiny
fold + |z| / sum(M).

dtypes: mask is 0/1 -> exact in fp8e4 (1 byte, halves HBM traffic, full PE rate);
weights/CS fp16 (PE full rate); SW in fp8 (the imag side is an incoherent sum,
tiny vs the coherent real part, so fp8 there costs ~1e-5 extra error);
PSUM/epilogue fp32. End-to-end rel err ~2e-5.
Column groups are sized small-big-small: a small first group starts the PE
early, small last groups shorten the end-of-stream epilogue tail. Trig rides
the scalar HWDGE ring (doesn't queue behind masks); a PE warm-up burst during
the DMA lead-in defeats the HAM cold-clock penalty.
"""

import numpy as np

import concourse.bass as bass
import concourse.tile as tile
from concourse import bacc, mybir
from concourse.bass_utils import run_bass_kernel_spmd

B = 64
NA = 8192
NB = 8192
NCORES = 8
NASH = NA // NCORES          # mask rows per core
KCH = NASH // 128            # contraction chunks of 128 rows
NCH = 512                    # output columns per PSUM bank / matmul

# column group widths: small first (early PE start), small last (short tail)
GWS = [512, 1024, 1024, 1024, 1024, 1024, 1024, 512, 512, 256, 256]
assert sum(GWS) == NB and all(w % 256 == 0 for w in GWS)
NG = len(GWS)
GOFF = [sum(GWS[:i]) for i in range(NG)]

# trig upload pieces (scalar ring): first small so group 0's epilogue isn't gated
TP = [1024, 2048, 2560, 2560]
assert sum(TP) == NB
TPOFF = [sum(TP[:i]) for i in range(len(TP))]

F8 = mybir.dt.float8e4
F16 = mybir.dt.float16
F32 = mybir.dt.float32


def build_program() -> bass.Bass:
    nc = bacc.Bacc("TRN2")
    # concatenated per-group blocks, each contiguous [128, KCH, gw]
    mask_d = nc.dram_tensor("mask", [128 * KCH * NB], F8, kind="ExternalInput")
    w_d = nc.dram_tensor("w", [128, KCH, 2 * B], F16, kind="ExternalInput")
    cs_d = nc.dram_tensor("cs", [128, NB], F16, kind="ExternalInput")
    sw_d = nc.dram_tensor("sw", [128, NB], F8, kind="ExternalInput")
    out_d = nc.dram_tensor("out", [128, 2 * NG], F32, kind="ExternalOutput")

    copy_f = mybir.ActivationFunctionType.Copy

    with tile.TileContext(nc) as tc:
        with (
            tc.tile_pool(name="consts", bufs=1) as consts,
            tc.tile_pool(name="masks", bufs=NG) as masks,
            tc.tile_pool(name="scratch", bufs=3) as scratch,
            tc.tile_pool(name="junk", bufs=2) as junkp,
            tc.tile_pool(name="psum", bufs=3, space="PSUM") as psum_pool,
            tc.tile_pool(name="wups", bufs=1, space="PSUM") as wu_pool,
        ):
            w_sb = consts.tile([128, KCH, 2 * B], F16)
            nc.sync.dma_start(out=w_sb[:], in_=w_d[:])
            cs_sb = consts.tile([128, NB], F16)
            sw_sb = consts.tile([128, NB], F8)
            racc = consts.tile([128, 2 * NG], F32)

            # PE warm-up while the first mask group is in flight (HAM ramp)
            wu_ps = wu_pool.tile([128, 2 * B], F32)
            for r in range(16):
                nc.tensor.matmul(
                    out=wu_ps[:],
                    lhsT=w_sb[:, 0, :],
                    rhs=w_sb[:, 1, :],
                    start=(r == 0),
                    stop=(r == 15),
                )

            tp_emitted = 0
            for g in range(NG):
                off, gw = GOFF[g], GWS[g]
                gsl = slice(off, off + gw)
                mt = masks.tile([128, KCH, gw], F8, tag="mask")
                blk = 128 * KCH
                src = mask_d[off * blk : (off + gw) * blk].rearrange(
                    "(p k c) -> p k c", p=128, k=KCH
                )
                nc.sync.dma_start(out=mt[:], in_=src)
                # trig pieces on the scalar HWDGE ring, paced ahead of use
                while tp_emitted < len(TP) and TPOFF[tp_emitted] < off + gw:
                    tsl = slice(TPOFF[tp_emitted], TPOFF[tp_emitted] + TP[tp_emitted])
                    nc.scalar.dma_start(out=cs_sb[:, tsl], in_=cs_d[:, tsl])
                    nc.scalar.dma_start(out=sw_sb[:, tsl], in_=sw_d[:, tsl])
                    tp_emitted += 1

                ps = psum_pool.tile([128, gw], F32, tag="psum")
                for j0 in range(0, gw, NCH):
                    jsl = slice(j0, min(j0 + NCH, gw))
                    for k in range(KCH):
                        nc.tensor.matmul(
                            out=ps[:, jsl],
                            lhsT=w_sb[:, k, :],
                            rhs=mt[:, k, jsl],
                            start=(k == 0),
                            stop=(k == KCH - 1),
                        )
                rcol = g if g < 8 else 16 + (g - 8)
                qcol = 8 + g if g < 8 else 16 + (NG - 8) + (g - 8)
                pr = scratch.tile([128, gw], F32, tag="pr")
                nc.vector.tensor_mul(out=pr[:], in0=ps[:], in1=cs_sb[:, gsl])
                jr = junkp.tile([128, gw], F32, tag="junk")
                nc.scalar.activation(
                    out=jr[:], in_=pr[:], func=copy_f,
                    accum_out=racc[:, rcol : rcol + 1],
                )
                pi = scratch.tile([128, gw], F32, tag="pr")
                nc.vector.tensor_mul(out=pi[:], in0=ps[:], in1=sw_sb[:, gsl])
                ji = junkp.tile([128, gw], F32, tag="junk")
                nc.scalar.activation(
                    out=ji[:], in_=pi[:], func=copy_f,
                    accum_out=racc[:, qcol : qcol + 1],
                )
                if g == 7:
                    # groups 0-7 partials fly out while tail groups finish
                    nc.sync.dma_start(out=out_d[:, :16], in_=racc[:, :16])

            nc.sync.dma_start(out=out_d[:, 16:], in_=racc[:, 16:])
    nc.finalize()
    return nc


def prep_inputs(phases_a, phases_b, coupling_mask):
    pa = np.asarray(phases_a, dtype=np.float32)
    pb = np.asarray(phases_b, dtype=np.float32)
    ca, sa = np.cos(pa), np.sin(pa)
    cb, sb = np.cos(pb), np.sin(pb)
    cs = np.concatenate([cb, sb], axis=0).astype(np.float16)
    sw = np.concatenate([sb, cb], axis=0).astype(mybir.dt.np(F8))

    f8np = mybir.dt.np(F8)
    one_byte = np.array([1.0], f8np).view(np.uint8)[0]
    mask_u8 = (np.asarray(coupling_mask) != 0).astype(np.uint8) * one_byte

    in_maps = []
    for c in range(NCORES):
        rows = slice(c * NASH, (c + 1) * NASH)
        W = np.empty((NASH, 2 * B), np.float16)
        W[:, :B] = ca[:, rows].T
        W[:, B:] = sa[:, rows].T
        # [i = k*128 + p, m] -> [p, k, m]
        w_host = np.ascontiguousarray(W.reshape(KCH, 128, 2 * B).transpose(1, 0, 2))
        # per group: contiguous [p, k, c] block; blocks concatenated flat
        mr = mask_u8[rows].reshape(KCH, 128, NB)
        blocks = [
            np.ascontiguousarray(
                mr[:, :, GOFF[g] : GOFF[g] + GWS[g]].transpose(1, 0, 2)
            ).reshape(-1)
            for g in range(NG)
        ]
        m_host = np.concatenate(blocks).view(f8np)
        in_maps.append({"mask": m_host, "w": w_host, "cs": cs, "sw": sw})
    return in_maps


def combine(outs, coupling_mask):
    o = np.stack(outs).astype(np.float64)  # [NCORES, 128, 2*NG]
    nt = NG - 8
    r = o[:, :, :8].sum(axis=2) + o[:, :, 16 : 16 + nt].sum(axis=2)
    q = o[:, :, 8:16].sum(axis=2) + o[:, :, 16 + nt :].sum(axis=2)
    real = (r[:, :B] + r[:, B:]).sum(axis=0)
    imag = (q[:, B:] - q[:, :B]).sum(axis=0)
    n_pairs = max(float(np.asarray(coupling_mask).sum()), 1.0)
    return (np.sqrt(real * real + imag * imag) / n_pairs).astype(np.float32)


_prog_cache: list = []


def kernel(phases_a, phases_b, coupling_mask):
    in_maps = prep_inputs(phases_a, phases_b, coupling_mask)
    if not _prog_cache:
        _prog_cache.append(build_program())
    res = run_bass_kernel_spmd(_prog_cache[0], in_maps, core_ids=list(range(NCORES)))
    return combine([r["out"] for r in res.results], coupling_mask)



# revision 3
# speedup vs baseline: 1.1589x; 1.1589x over previous
"""Masked phase-locking value (PLV) kernel for Trainium2, 8 NeuronCores.

Math: out[b] = |sum_ij M_ij * exp(i*(a_bi - b_bj))| / max(sum(M), 1)
    real_b = sum_ij M_ij (cos a_bi cos b_bj + sin a_bi sin b_bj)
    imag_b = sum_ij M_ij (sin a_bi cos b_bj - cos a_bi sin b_bj)

Device decomposition (per core, Na sharded 8 ways -> NI=1024 rows each):
    Y[m, i] = sum_j V[j, m] * M[i, j]      (TensorE, fp8 DoubleRow; V = [cb^T | sb^T], m = 2B = 128)
    racc[m] = sum_i Y[m, i] * U[m, i]      (DVE fused tensor_tensor_reduce; U = [ca;sa] / [sa;-ca])
real_b = sum_cores racc_r[b] + racc_r[64+b]; imag_b likewise from racc_q.
Bilinear in mask rows, so Na-shard partials just add; host does the tiny
fold + |z| / sum(M).

Key perf structure (kernel is mask-DMA-bound, ~8MB/core at ~360GB/s):
  - matmul flipped vs naive (trig stationary, mask moving): PSUM output is
    [128, 1024] instead of [128, 8192] -> 8x smaller epilogue.
  - fp8 DoubleRow matmul: 256-row contraction per instruction, ~1.5-2x PE rate;
    mask is 0/1 -> exact in fp8; trig weights fp8 (rel err ~2.5e-3 << 2e-2).
  - mask DMA: p-major host layout -> contiguous >=2KB/partition descriptors,
    streamed in blocks on the sync HWDGE ring with no interleaved stalls;
    trig/U on the scalar ring in parallel. First blocks small for fast start.
  - PE warm-up on memset junk (no DMA dependency) defeats the cold-clock ramp.
  - epilogue per i-slice fires as soon as that slice's accumulation closes;
    partial results DMA out early, only the last slice's reduce is tail.
"""

import numpy as np

import concourse.bass as bass
import concourse.tile as tile
from concourse import bacc, mybir
from concourse.bass_utils import run_bass_kernel_spmd

B = 64
NA = 8192
NB = 8192
NCORES = 8
NI = NA // NCORES            # mask rows (i) per core
KC = NB // 256               # 32 contraction chunks of 256 j-rows
TK = 2 * KC                  # tile dim1: t = 2k + q (DoubleRow pair slot q)
ISL = 256                    # i-slice width (PSUM half-bank)
NSL = NI // ISL              # 4 i-slices
NWU = 16                     # PE warm-up matmuls
# mask DMA blocks in k-chunks (256KB each): small first for fast PE start
MBLK_K = [1, 1, 2, 4, 4, 4, 4, 4, 4, 4]
assert sum(MBLK_K) == KC

F8 = mybir.dt.float8e4
F16 = mybir.dt.float16
F32 = mybir.dt.float32


def build_program() -> bass.Bass:
    nc = bacc.Bacc("TRN2")
    # host layouts are p-major: dim0 = SBUF partition, per-partition contiguous
    mask_d = nc.dram_tensor("mask", [128, TK, NI], F8, kind="ExternalInput")
    v_d = nc.dram_tensor("v", [128, TK, 2 * B], F8, kind="ExternalInput")
    u_d = nc.dram_tensor("u", [128, 2, NI], F16, kind="ExternalInput")
    out_d = nc.dram_tensor("out", [128, 2 * NSL], F32, kind="ExternalOutput")

    DR = mybir.MatmulPerfMode.DoubleRow
    mult = mybir.AluOpType.mult
    add = mybir.AluOpType.add

    with tile.TileContext(nc) as tc:
        with (
            tc.tile_pool(name="consts", bufs=1) as consts,
            tc.tile_pool(name="psum", bufs=1, space="PSUM") as psum_pool,
            tc.tile_pool(name="wups", bufs=1, space="PSUM") as wu_pool,
        ):
            jw = consts.tile([128, 2, 256], F8)
            nc.vector.memset(jw, 0)
            mask_sb = consts.tile([128, TK, NI], F8)
            v_sb = consts.tile([128, TK, 2 * B], F8)
            u_sb = consts.tile([128, 2, NI], F16)
            racc = consts.tile([128, 2 * NSL], F32)
            jr = consts.tile([128, ISL], F32)

            # scalar HWDGE ring: trig weights (head first - gates k=0), then U
            nc.scalar.dma_start(out=v_sb[:, 0:8], in_=v_d[:, 0:8])
            nc.scalar.dma_start(out=v_sb[:, 8:TK], in_=v_d[:, 8:TK])
            nc.scalar.dma_start(out=u_sb[:], in_=u_d[:])
            # sync HWDGE ring: mask blocks, issued back-to-back with no waits
            k0 = 0
            for nk in MBLK_K:
                tsl = slice(2 * k0, 2 * (k0 + nk))
                nc.sync.dma_start(out=mask_sb[:, tsl], in_=mask_d[:, tsl])
                k0 += nk

            # PE warm-up on junk (no DMA dependency) to beat the clock ramp
            wu = wu_pool.tile([128, 256], F32)
            for r in range(NWU):
                nc.tensor.matmul(
                    out=wu[:], lhsT=jw[:, :, 0:128], rhs=jw[:],
                    start=(r == 0), stop=(r == NWU - 1), perf_mode=DR,
                )

            ps = psum_pool.tile([128, NI], F32)
            for k in range(KC):
                tsl = slice(2 * k, 2 * k + 2)
                for s in range(NSL):
                    isl = slice(s * ISL, (s + 1) * ISL)
                    nc.tensor.matmul(
                        out=ps[:, isl],
                        lhsT=v_sb[:, tsl, :],
                        rhs=mask_sb[:, tsl, isl],
                        start=(k == 0), stop=(k == KC - 1), perf_mode=DR,
                    )
                    if k == KC - 1:
                        # this slice's accumulation is closed: reduce it now,
                        # overlapping the remaining slices' matmuls
                        for h in (0, 1):
                            col = 2 * s + h
                            nc.vector.scalar_tensor_tensor(
                                out=jr[:], in0=ps[:, isl], scalar=1.0,
                                in1=u_sb[:, h, isl], op0=mult, op1=mult,
                                accum_out=racc[:, col : col + 1],
                            )
                        if s == NSL - 2:
                            # first slices' results fly out under the tail
                            nc.sync.dma_start(
                                out=out_d[:, : 2 * (NSL - 1)],
                                in_=racc[:, : 2 * (NSL - 1)],
                            )
            nc.sync.dma_start(
                out=out_d[:, 2 * (NSL - 1) :], in_=racc[:, 2 * (NSL - 1) :]
            )
    nc.finalize()
    return nc


def prep_inputs(phases_a, phases_b, coupling_mask):
    f8np = mybir.dt.np(F8)
    pa = np.asarray(phases_a, dtype=np.float32)
    pb = np.asarray(phases_b, dtype=np.float32)
    ca, sa = np.cos(pa), np.sin(pa)
    cb, sb = np.cos(pb), np.sin(pb)

    one_byte = np.array([1.0], f8np).view(np.uint8)[0]
    mask_u8 = (np.asarray(coupling_mask) != 0).astype(np.uint8) * one_byte

    # V[p, t=2k+q, m]: trig value for j = 256k + 2p + q, m = batch (cb|sb)
    T2 = np.concatenate([cb, sb], axis=0)                      # [128 m, NB j]
    v_host = (
        np.ascontiguousarray(T2.T.reshape(KC, 128, 2, 2 * B).transpose(1, 0, 2, 3))
        .reshape(128, TK, 2 * B)
        .astype(f8np)
    )

    in_maps = []
    for c in range(NCORES):
        sl = slice(c * NI, (c + 1) * NI)
        A = mask_u8[sl]                                        # [NI i, NB j]
        m_host = (
            np.ascontiguousarray(A.reshape(NI, KC, 128, 2).transpose(2, 1, 3, 0))
            .reshape(128, TK, NI)
            .view(f8np)
        )
        u_host = np.stack(
            [
                np.concatenate([ca[:, sl], sa[:, sl]], axis=0),
                np.concatenate([sa[:, sl], -ca[:, sl]], axis=0),
            ],
            axis=1,
        ).astype(np.float16)                                   # [128, 2, NI]
        in_maps.append({"mask": m_host, "v": v_host, "u": u_host})
    return in_maps


def combine(outs, coupling_mask):
    o = np.stack(outs).astype(np.float64)      # [NCORES, 128, 2*NSL]
    r = o[:, :, 0::2].sum(axis=(0, 2))         # [128]
    q = o[:, :, 1::2].sum(axis=(0, 2))
    real = r[:B] + r[B:]
    imag = q[:B] + q[B:]
    n_pairs = max(float(np.count_nonzero(np.asarray(coupling_mask))), 1.0)
    return (np.sqrt(real * real + imag * imag) / n_pairs).astype(np.float32)


_prog_cache: list = []


def kernel(phases_a, phases_b, coupling_mask):
    in_maps = prep_inputs(phases_a, phases_b, coupling_mask)
    if not _prog_cache:
        _prog_cache.append(build_program())
    res = run_bass_kernel_spmd(_prog_cache[0], in_maps, core_ids=list(range(NCORES)))
    return combine([r["out"] for r in res.results], coupling_mask)


# revision 7
# speedup vs baseline: 1.2409x; 1.0708x over previous
"""Masked phase-locking value (PLV) kernel for Trainium2, 8 NeuronCores.

Math: out[b] = |sum_ij M_ij * exp(i*(a_bi - b_bj))| / max(sum(M), 1)
    real_b = sum_ij M_ij (cos a_bi cos b_bj + sin a_bi sin b_bj)
    imag_b = sum_ij M_ij (sin a_bi cos b_bj - cos a_bi sin b_bj)

Device decomposition (per core, Na sharded 8 ways -> NI=1024 rows each):
    Y[m, i] = sum_j V[j, m] * M[i, j]      (TensorE, fp8 DoubleRow; V = [cb^T | sb^T], m = 2B = 128)
    racc[m] = sum_i Y[m, i] * U[m, i]      (DVE fused scalar_tensor_tensor; U = [ca;sa] / [sa;-ca])
real_b = sum_cores racc_r[b] + racc_r[64+b]; imag_b likewise from racc_q.
Bilinear in mask rows, so Na-shard partials just add; host does the tiny
fold + |z| / sum(M).

The kernel is HBM-bound: ~9.25MB/core (8MB mask fp8 + 1MB trig weights fp8
+ 0.25MB U fp8) at the ~330GB/s per-core share of chip HBM. Everything else
hides under the mask stream:
  - matmul flipped vs naive (trig stationary, mask moving): PSUM output is
    [128, 1024] instead of [128, 8192] -> 8x smaller epilogue.
  - fp8 DoubleRow matmul (contraction 256/instr, FD=512): ~1.5x PE rate;
    mask is 0/1 -> exact in fp8; trig fp8 (end-to-end rel err ~3e-3 << 2e-2).
  - each matmul accumulation region owns a full PSUM bank: start_tensor_calc
    zeroes the whole bank row, so regions must never share a bank.
  - mask streamed on the sync HWDGE ring as few large blocks (descriptor-gen
    and per-transfer completion latency amortized), tiny last block so the
    final completion semaphore gates almost no work; trig on the scalar ring.
  - PE warm-up on memset junk (no DMA dependency) defeats the cold-clock ramp.
  - epilogue (one fused DVE op per i-slice half) fires per bank as soon as
    that bank's accumulation closes; first results DMA out under the tail.
"""

import numpy as np

import concourse.bass as bass
import concourse.tile as tile
from concourse import bacc, mybir
from concourse.bass_utils import run_bass_kernel_spmd

B = 64
NA = 8192
NB = 8192
NCORES = 8
NI = NA // NCORES            # mask rows (i) per core
KC = NB // 256               # 32 contraction chunks of 256 j-rows
TK = 2 * KC                  # tile dim1: t = 2k + q (DoubleRow pair slot q)
MMSL = 256                   # matmul i-slice (FD); each owns a full PSUM bank
NBK = NI // MMSL             # 4 accumulation banks
ESL = 256                    # epilogue i-slice
NSL = NI // ESL              # 4 epilogue slices
NWU = 12                     # PE warm-up matmuls
# mask DMA blocks in k-chunks (256KB each): large early for stream
# continuity, tiny last so the final completion sem gates minimal work
MBLK_K = [4, 8, 8, 8, 3, 1]
assert sum(MBLK_K) == KC

F8 = mybir.dt.float8e4
F16 = mybir.dt.float16
F32 = mybir.dt.float32


def build_program() -> bass.Bass:
    nc = bacc.Bacc("TRN2")
    # host layouts are p-major: dim0 = SBUF partition, per-partition contiguous
    mask_d = nc.dram_tensor("mask", [128, TK, NI], F8, kind="ExternalInput")
    v_d = nc.dram_tensor("v", [128, TK, 2 * B], F8, kind="ExternalInput")
    u_d = nc.dram_tensor("u", [128, 2, NI], F8, kind="ExternalInput")
    out_d = nc.dram_tensor("out", [128, 2 * NSL], F32, kind="ExternalOutput")

    DR = mybir.MatmulPerfMode.DoubleRow
    mult = mybir.AluOpType.mult

    with tile.TileContext(nc) as tc:
        with (
            tc.tile_pool(name="consts", bufs=1) as consts,
            tc.tile_pool(name="psum", bufs=1, space="PSUM") as psum_pool,
        ):
            jw = consts.tile([128, 2, MMSL], F8)
            nc.vector.memset(jw, 0)
            mask_sb = consts.tile([128, TK, NI], F8)
            v_sb = consts.tile([128, TK, 2 * B], F8)
            u_sb = consts.tile([128, 2, NI], F8)
            racc = consts.tile([128, 2 * NSL], F32)
            jr = consts.tile([128, ESL], F32)

            # scalar HWDGE ring: trig weights (gate k=0), then epilogue U
            nc.scalar.dma_start(out=v_sb[:], in_=v_d[:])
            nc.scalar.dma_start(out=u_sb[:], in_=u_d[:])
            # sync HWDGE ring: mask blocks, issued back-to-back with no waits
            k0 = 0
            for nk in MBLK_K:
                tsl = slice(2 * k0, 2 * (k0 + nk))
                nc.sync.dma_start(out=mask_sb[:, tsl], in_=mask_d[:, tsl])
                k0 += nk

            # full-bank PSUM tiles: one accumulation region per bank
            # (start_tensor_calc zeroes the whole bank row)
            pss = [
                psum_pool.tile([128, 512], F32, name=f"ps{i}") for i in range(NBK)
            ]
            wu = psum_pool.tile([128, 512], F32)

            # PE warm-up on junk (no DMA dependency) to beat the clock ramp
            for r in range(NWU):
                nc.tensor.matmul(
                    out=wu[:, 0:MMSL], lhsT=jw[:, :, 0:128], rhs=jw[:],
                    start=(r == 0), stop=(r == NWU - 1), perf_mode=DR,
                )

            for k in range(KC):
                tsl = slice(2 * k, 2 * k + 2)
                for sb in range(NBK):
                    msl = slice(sb * MMSL, (sb + 1) * MMSL)
                    nc.tensor.matmul(
                        out=pss[sb][:, 0:MMSL],
                        lhsT=v_sb[:, tsl, :],
                        rhs=mask_sb[:, tsl, msl],
                        start=(k == 0), stop=(k == KC - 1), perf_mode=DR,
                    )
                    if k == KC - 1:
                        # bank closed: fused multiply+reduce, overlapping
                        # the remaining banks' matmuls
                        for h in (0, 1):
                            col = 2 * sb + h
                            nc.vector.scalar_tensor_tensor(
                                out=jr[:], in0=pss[sb][:, 0:MMSL], scalar=1.0,
                                in1=u_sb[:, h, sb * MMSL : (sb + 1) * MMSL],
                                op0=mult, op1=mult,
                                accum_out=racc[:, col : col + 1],
                            )
                        if sb == NBK - 2:
                            # early banks' results fly out under the tail
                            nc.sync.dma_start(
                                out=out_d[:, : 2 * (NBK - 1)],
                                in_=racc[:, : 2 * (NBK - 1)],
                            )
            nc.sync.dma_start(
                out=out_d[:, 2 * (NBK - 1) :], in_=racc[:, 2 * (NBK - 1) :]
            )
    nc.finalize()
    return nc


def prep_inputs(phases_a, phases_b, coupling_mask):
    f8np = mybir.dt.np(F8)
    pa = np.asarray(phases_a, dtype=np.float32)
    pb = np.asarray(phases_b, dtype=np.float32)
    ca, sa = np.cos(pa), np.sin(pa)
    cb, sb = np.cos(pb), np.sin(pb)

    one_byte = np.array([1.0], f8np).view(np.uint8)[0]
    mask_u8 = (np.asarray(coupling_mask) != 0).astype(np.uint8) * one_byte

    # V[p, t=2k+q, m]: trig value for j = 256k + 2p + q, m = batch (cb|sb)
    T2 = np.concatenate([cb, sb], axis=0)                      # [128 m, NB j]
    v_host = (
        np.ascontiguousarray(T2.T.reshape(KC, 128, 2, 2 * B).transpose(1, 0, 2, 3))
        .reshape(128, TK, 2 * B)
        .astype(f8np)
    )

    in_maps = []
    for c in range(NCORES):
        sl = slice(c * NI, (c + 1) * NI)
        A = mask_u8[sl]                                        # [NI i, NB j]
        m_host = (
            np.ascontiguousarray(A.reshape(NI, KC, 128, 2).transpose(2, 1, 3, 0))
            .reshape(128, TK, NI)
            .view(f8np)
        )
        u_host = np.stack(
            [
                np.concatenate([ca[:, sl], sa[:, sl]], axis=0),
                np.concatenate([sa[:, sl], -ca[:, sl]], axis=0),
            ],
            axis=1,
        ).astype(f8np)                                         # [128, 2, NI]
        in_maps.append({"mask": m_host, "v": v_host, "u": u_host})
    return in_maps


def combine(outs, coupling_mask):
    o = np.stack(outs).astype(np.float64)      # [NCORES, 128, 2*NSL]
    r = o[:, :, 0::2].sum(axis=(0, 2))         # [128]
    q = o[:, :, 1::2].sum(axis=(0, 2))
    real = r[:B] + r[B:]
    imag = q[:B] + q[B:]
    n_pairs = max(float(np.count_nonzero(np.asarray(coupling_mask))), 1.0)
    return (np.sqrt(real * real + imag * imag) / n_pairs).astype(np.float32)


_prog_cache: list = []


def kernel(phases_a, phases_b, coupling_mask):
    in_maps = prep_inputs(phases_a, phases_b, coupling_mask)
    if not _prog_cache:
        _prog_cache.append(build_program())
    res = run_bass_kernel_spmd(_prog_cache[0], in_maps, core_ids=list(range(NCORES)))
    return combine([r["out"] for r in res.results], coupling_mask)
